# revision 30
# baseline (speedup 1.0000x reference)
"""TRN2 8-core SPMD kernel for nn_DecoderBlock_13443247636967 (v4).

Math (validated vs fp32 reference in numpy, rel err ~8.5e-3):
the reference's softmax scale HS**-5 = 2**-30 makes every pre-softmax
score < 4e-8, so softmax is exactly the uniform causal average at fp32
resolution and attention reduces to a causal prefix-mean of V.  Since
prefix-mean is linear, attn_out = prefix_mean(x) @ (Wv @ Wo): the value
and output projections fold into one host-precomputed matrix Wvo and V
is never materialized.  Wk cannot affect the output at fp32 resolution.

Device pipeline per 128-row tile (core c = batch c//2, half c%2):
  P^T = matmul(lhsT=xp-chunk, rhs=count-scaled-upper-tri)  [8 matmuls;
        the cross-tile carry (cumulative column sum of x) is folded by
        the host into row 0 of xp, which the inclusive triangular
        matrix then propagates to every row]
  AO  = P^T(fp8) @ Wvo(fp8)            [DoubleRow, 2x PE rate]
  r1  = AO*0.25 + x256                 [fused scalar_tensor_tensor]
  N1  = LN1(r1)  (scale-invariant => the 256x scaling cancels)
  H   = relu(N1T(fp8) @ Wf1(fp8))      [DoubleRow]
  z   = HT(fp8) @ Wf2(fp8) + (N1 + x)  [N1+x add runs on idle GpSimd]
  out = LN2(z) -> bf16, host casts to fp32.

Scales: weights host-prescaled by 64 (lifts them out of fp8e4
subnormals); activations carry powers-of-two scales (x: 256, P^T: 16,
N1T: 16, H: 4) folded into the PSUM->SBUF cast constants and LayerNorm
coefficient algebra.  Transposes run on the PE in bf16; quantization
to fp8 happens in the contiguous PSUM->SBUF cast.
"""

import numpy as np
import ml_dtypes

import concourse.bass as bass
import concourse.mybir as mybir
import concourse.tile as tile
from concourse import bacc
from concourse.bass_utils import run_bass_kernel_spmd
from concourse.masks import make_identity

P = 128          # partitions / row-tile height
D = 1024         # model dim
TH = 1024        # sequence rows per core
NT = TH // P     # 8 row tiles
KC = D // P      # 8 contraction chunks
NF = 512         # psum bank free dim (fp32)
NH = D // NF     # 2 column halves
B, T = 4, 2048
F32 = mybir.dt.float32
BF = mybir.dt.bfloat16
F8 = mybir.dt.float8e4
DR = mybir.MatmulPerfMode.DoubleRow
AF = mybir.ActivationFunctionType
ALU = mybir.AluOpType


def _build(lean=True):
    nc = bacc.Bacc(
        "TRN2", target_bir_lowering=False, debug=False, num_devices=8
    )
    x256 = nc.dram_tensor("x256", [TH, D], BF, kind="ExternalInput").ap()
    xp256 = nc.dram_tensor("xp256", [TH, D], BF, kind="ExternalInput").ap()
    ut16 = nc.dram_tensor("ut16", [P, NT, P], BF, kind="ExternalInput").ap()
    Wvo8 = nc.dram_tensor("Wvo8", [P, KC, D], F8, kind="ExternalInput").ap()
    Wf18 = nc.dram_tensor("Wf18", [P, KC, D], F8, kind="ExternalInput").ap()
    Wf28 = nc.dram_tensor("Wf28", [P, KC, D], F8, kind="ExternalInput").ap()
    if not lean:
        xb256 = nc.dram_tensor("xb256", [TH, D], BF, kind="ExternalInput").ap()
        gb = nc.dram_tensor("gb", [4, D], BF, kind="ExternalInput").ap()
        fb = nc.dram_tensor("fb", [2, 2, D], BF, kind="ExternalInput").ap()
    out_bf = nc.dram_tensor("out_bf", [TH, D], BF, kind="ExternalOutput").ap()

    with tile.TileContext(nc) as tc:
        with tc.tile_pool(name="w", bufs=3) as wpool, \
             tc.tile_pool(name="cn", bufs=1) as cn, \
             tc.tile_pool(name="xs", bufs=3) as xpool, \
             tc.tile_pool(name="xp", bufs=3) as xppool, \
             tc.tile_pool(name="r1", bufs=3) as r1pool, \
             tc.tile_pool(name="nx", bufs=3) as nxpool, \
             tc.tile_pool(name="n8", bufs=3) as n8pool, \
             tc.tile_pool(name="h8", bufs=3) as h8pool, \
             tc.tile_pool(name="tp", bufs=4) as tpool, \
             tc.tile_pool(name="p8", bufs=3) as p8pool, \
             tc.tile_pool(name="z", bufs=3) as zpool, \
             tc.tile_pool(name="o", bufs=3) as opool, \
             tc.tile_pool(name="st", bufs=3) as stat, \
             tc.tile_pool(name="pfx", bufs=2, space="PSUM") as pfx, \
             tc.tile_pool(name="pao", bufs=2, space="PSUM") as pao, \
             tc.tile_pool(name="pf", bufs=2, space="PSUM") as pf, \
             tc.tile_pool(name="ptp", bufs=2, space="PSUM") as ptp:

            # ---- constants / weights ----
            identb = cn.tile([P, P], BF)
            make_identity(nc, identb)
            eps1 = cn.tile([P, 1], F32)
            nc.vector.memset(eps1, 1e-5)
            eps2 = cn.tile([P, 1], F32)
            nc.vector.memset(eps2, 0.65536)
            # warm the scalar engine's activation tables while DMAs run
            warm = cn.tile([P, 1], F32)
            nc.scalar.activation(
                out=warm, in_=eps1, func=AF.Sqrt, bias=eps2, scale=1.0)
            nc.scalar.activation(
                out=warm, in_=eps1, func=AF.Copy, scale=1.0)
            ut_sb = cn.tile([P, NT, P], BF)
            nc.sync.dma_start(out=ut_sb, in_=ut16)

            def load_w(ap, name):
                # two half-DMAs: the first AO column group only needs
                # its half of the weights to have landed
                w = wpool.tile([P, KC, D], F8, tag="W", name=name)
                for n in range(NH):
                    nsl = slice(n * NF, (n + 1) * NF)
                    nc.sync.dma_start(out=w[:, :, nsl], in_=ap[:, :, nsl])
                return w

            Wvo_sb = load_w(Wvo8, "Wvo")
            Wf1_sb = load_w(Wf18, "Wf1")
            Wf2_sb = load_w(Wf28, "Wf2")

            if not lean:
                g1_bc = cn.tile([P, D], BF)
                nc.sync.dma_start(
                    out=g1_bc, in_=gb[0:1, :].to_broadcast([P, D]))
                g2_bc = cn.tile([P, D], BF)
                nc.sync.dma_start(
                    out=g2_bc, in_=gb[1:2, :].to_broadcast([P, D]))
                b2_bc = cn.tile([P, D], BF)
                nc.sync.dma_start(
                    out=b2_bc, in_=gb[2:3, :].to_broadcast([P, D]))
                b1_bc = cn.tile([P, D], BF)
                nc.sync.dma_start(
                    out=b1_bc, in_=gb[3:4, :].to_broadcast([P, D]))
                fb_sb = cn.tile([2, 2, D], BF)
                nc.sync.dma_start(out=fb_sb, in_=fb)
                ones2 = cn.tile([2, P], BF)
                nc.vector.memset(ones2[0:1, :], 1.0)
                nc.vector.memset(ones2[1:2, :], 0.0)

            def ln_coeffs(src, eps_t, scale, tag):
                """bn stats + rstd/bias for ACT apply: (src - m) * q."""
                st = stat.tile([P, NH, 6], F32, tag=f"st{tag}")
                for h in range(NH):
                    nc.vector.bn_stats(
                        out=st[:, h, :], in_=src[:, h * NF:(h + 1) * NF])
                mv = stat.tile([P, 2], F32, tag=f"mv{tag}")
                nc.vector.bn_aggr(out=mv, in_=st)
                s = stat.tile([P, 1], F32, tag=f"s{tag}")
                nc.scalar.activation(
                    out=s, in_=mv[:, 1:2], func=AF.Sqrt,
                    bias=eps_t, scale=scale,
                )
                q = stat.tile([P, 1], F32, tag=f"q{tag}")
                nc.vector.reciprocal(out=q, in_=s)
                mb = stat.tile([P, 1], F32, tag=f"mb{tag}")
                nc.vector.tensor_scalar(
                    out=mb, in0=mv[:, 0:1], scalar1=q, scalar2=-1.0,
                    op0=ALU.mult, op1=ALU.mult,
                )
                return q, mb

            for j in range(NT):
                jsl = slice(j * P, (j + 1) * P)
                x_t = xpool.tile([P, D], BF, tag="x", name="x")
                nc.sync.dma_start(out=x_t, in_=x256[jsl, :])
                xp_t = xppool.tile([P, D], BF, tag="xp", name="xp")
                nc.sync.dma_start(out=xp_t, in_=xp256[jsl, :])
                if not lean:
                    xb_t = xpool.tile([P, D], BF, tag="xb", name="xb")
                    nc.sync.dma_start(out=xb_t, in_=xb256[jsl, :])

                # ---- prefix-mean, transposed: pt[d, t] = 4096 * P^T ----
                pt8 = p8pool.tile([P, KC, P], F8, tag="p8", name="PT8")
                for g in range(2):
                    pts = pfx.tile([P, 4, P], F32, tag="pf", name="pts")
                    for k4 in range(4):
                        kc = 4 * g + k4
                        nc.tensor.matmul(
                            pts[:, k4, :],
                            lhsT=xp_t[:, kc * P:(kc + 1) * P],
                            rhs=ut_sb[:, j, :],
                            start=True, stop=True,
                        )
                    nc.scalar.activation(
                        out=pt8[:, 4 * g:4 * g + 4, :], in_=pts,
                        func=AF.Copy, scale=1.0 / 256.0,
                    )

                # ---- AO = 16P @ 64Wvo = 1024*AO;  r1 = 256*(AO + x) ----
                r1 = r1pool.tile([P, D], BF, tag="r1", name="r1")
                for n in range(NH):
                    nsl = slice(n * NF, (n + 1) * NF)
                    ao = pao.tile([P, NF], F32, tag="ao")
                    for g in range(4):
                        gsl = slice(2 * g, 2 * g + 2)
                        nc.tensor.matmul(
                            ao, lhsT=pt8[:, gsl, :],
                            rhs=Wvo_sb[:, gsl, nsl],
                            start=(g == 0), stop=(g == 3), perf_mode=DR,
                        )
                    nc.vector.scalar_tensor_tensor(
                        out=r1[:, nsl], in0=ao, scalar=0.25,
                        in1=(x_t if lean else xb_t)[:, nsl],
                        op0=ALU.mult, op1=ALU.add,
                    )

                # ---- LN1: n18b = 256*N1 ; n1x = 256*(N1 + x) ----
                q1, mb1 = ln_coeffs(r1, eps1, 1.0 / 65536.0, "1")
                n18 = n8pool.tile([P, D], BF, tag="n8", name="N18")
                nc.scalar.activation(
                    out=n18, in_=r1, func=AF.Identity, bias=mb1, scale=q1)
                if not lean:
                    nc.vector.tensor_mul(out=n18, in0=n18, in1=g1_bc)
                n1x = nxpool.tile([P, D], BF, tag="nx", name="N1x")
                nc.gpsimd.tensor_add(out=n1x, in0=n18, in1=x_t)
                if not lean:
                    nc.gpsimd.tensor_add(out=n1x, in0=n1x, in1=b1_bc)

                # ---- FFN1: psum = 16N1 @ 64Wf1; h8 = 4*relu(N1@Wf1) ----
                t1 = ptp.tile([P, KC, P], BF, tag="tp")
                for kc in range(KC):
                    nc.tensor.transpose(
                        t1[:, kc, :], n18[:, kc * P:(kc + 1) * P], identb)
                n1t = tpool.tile([P, KC, P], F8, tag="t", name="N1T")
                nc.scalar.activation(
                    out=n1t, in_=t1, func=AF.Copy, scale=1.0 / 16.0)

                h8 = h8pool.tile([P, D], BF, tag="h8", name="H8")
                for n in range(NH):
                    nsl = slice(n * NF, (n + 1) * NF)
                    f1 = pf.tile([P, NF], F32, tag="f")
                    for g in range(4):
                        gsl = slice(2 * g, 2 * g + 2)
                        nc.tensor.matmul(
                            f1, lhsT=n1t[:, gsl, :],
                            rhs=Wf1_sb[:, gsl, nsl],
                            start=(g == 0), stop=(g == 3 and lean),
                            perf_mode=DR,
                        )
                    if not lean:
                        nc.tensor.matmul(
                            f1, lhsT=ones2, rhs=fb_sb[:, 0, nsl],
                            start=False, stop=True,
                        )
                    nc.scalar.activation(
                        out=h8[:, nsl], in_=f1, func=AF.Relu,
                        scale=1.0 / 256.0,
                    )

                # ---- FFN2: z = 256*(ff + N1 + x) ----
                t2 = ptp.tile([P, KC, P], BF, tag="tp")
                for kc in range(KC):
                    nc.tensor.transpose(
                        t2[:, kc, :], h8[:, kc * P:(kc + 1) * P], identb)
                ht = tpool.tile([P, KC, P], F8, tag="t", name="HT")
                nc.vector.tensor_copy(out=ht, in_=t2)

                z = zpool.tile([P, D], BF, tag="z", name="z")
                for n in range(NH):
                    nsl = slice(n * NF, (n + 1) * NF)
                    f2 = pf.tile([P, NF], F32, tag="f")
                    for g in range(4):
                        gsl = slice(2 * g, 2 * g + 2)
                        nc.tensor.matmul(
                            f2, lhsT=ht[:, gsl, :],
                            rhs=Wf2_sb[:, gsl, nsl],
                            start=(g == 0), stop=(g == 3 and lean),
                            perf_mode=DR,
                        )
                    if not lean:
                        nc.tensor.matmul(
                            f2, lhsT=ones2, rhs=fb_sb[:, 1, nsl],
                            start=False, stop=True,
                        )
                    nc.vector.tensor_add(
                        out=z[:, nsl], in0=f2, in1=n1x[:, nsl])

                # ---- LN2 -> out ----
                q2, mb2 = ln_coeffs(z, eps2, 1.0, "2")
                o = opool.tile([P, D], BF, tag="o", name="o")
                if lean:
                    nc.scalar.activation(
                        out=o, in_=z, func=AF.Identity, bias=mb2, scale=q2)
                else:
                    op = opool.tile([P, D], BF, tag="op", name="op")
                    nc.scalar.activation(
                        out=op, in_=z, func=AF.Identity, bias=mb2, scale=q2)
                    nc.vector.tensor_mul(out=op, in0=op, in1=g2_bc)
                    nc.vector.tensor_add(out=o, in0=op, in1=b2_bc)
                nc.sync.dma_start(out=out_bf[jsl, :], in_=o)

    nc.compile()
    return nc


_CACHE = {}


def _get_nc(lean=True):
    key = "lean" if lean else "general"
    if key not in _CACHE:
        _CACHE[key] = _build(lean=lean)
    return _CACHE[key]


BF_NP = ml_dtypes.bfloat16
F8_NP = ml_dtypes.float8_e4m3


def _in_maps(x, Wv, Wo, bo, g1, b1, Wf1, bf1, Wf2, bf2, g2, b2, lean=True):
    x = np.asarray(x, dtype=np.float32)
    Wv_all = np.asarray(Wv, np.float32).transpose(1, 0, 2).reshape(D, D)
    Wvo = Wv_all @ np.asarray(Wo, np.float32)

    def wprep(w):
        # [D, D] -> [P, KC, D] so each partition's SBUF row is contiguous
        w8 = np.asarray(64.0 * w, F8_NP)
        return np.ascontiguousarray(
            w8.reshape(KC, P, D).transpose(1, 0, 2))

    base = {
        "Wvo8": wprep(Wvo),
        "Wf18": wprep(np.asarray(Wf1, np.float32)),
        "Wf28": wprep(np.asarray(Wf2, np.float32)),
    }
    if not lean:
        base["gb"] = np.asarray(np.stack([
            np.asarray(g1, np.float32),
            np.asarray(g2, np.float32),
            np.asarray(b2, np.float32),
            256.0 * np.asarray(b1, np.float32),
        ]), BF_NP)
        base["fb"] = np.asarray(np.stack([
            np.stack([1024.0 * (np.asarray(b1, np.float32) @
                                np.asarray(Wf1, np.float32) +
                                np.asarray(bf1, np.float32)),
                      np.zeros(D, np.float32)]),
            np.stack([256.0 * np.asarray(bf2, np.float32),
                      np.zeros(D, np.float32)]),
        ]), BF_NP)

    # per-half triangular tables: ut[u, j, t] = 16/cnt_t for u <= t
    uts = []
    for half in range(2):
        t0 = half * TH
        ut = np.zeros((P, NT, P), np.float32)
        for j in range(NT):
            cnt = (t0 + j * P + np.arange(P) + 1.0).astype(np.float32)
            ut[:, j, :] = np.triu(np.ones((P, P), np.float32)) * (16.0 / cnt)
        uts.append(np.asarray(ut, BF_NP))

    in_maps = []
    for c in range(8):
        b, half = divmod(c, 2)
        t0 = half * TH
        m = dict(base)
        xh = x[b, t0:t0 + TH]
        m["x256"] = np.asarray(256.0 * xh, BF_NP)
        if not lean:
            m["xb256"] = np.asarray(
                256.0 * (xh + np.asarray(bo, np.float32)), BF_NP)
        # xp = x with the cumulative column-sum carry folded into each
        # tile's first row (the inclusive triangular matrix propagates
        # row 0 to every row of the tile)
        ts_sums = x[b].reshape(2 * NT, P, D).sum(axis=1, dtype=np.float64)
        starts = np.zeros((2 * NT, D), np.float64)
        starts[1:] = np.cumsum(ts_sums[:-1], axis=0)
        xp = 256.0 * xh.astype(np.float64)
        for j in range(NT):
            xp[j * P] += 256.0 * starts[half * NT + j]
        m["xp256"] = np.asarray(xp.astype(np.float32), BF_NP)
        m["ut16"] = uts[half]
        in_maps.append(m)
    return in_maps


def _assemble(results):
    out = np.empty((B, T, D), np.float32)
    for c in range(8):
        b, half = divmod(c, 2)
        out[b, half * TH:(half + 1) * TH] = (
            results[c]["out_bf"].astype(np.float32))
    return out


def kernel(x, Wk, Wv, Wo, bo, g1, b1, Wf1, bf1, Wf2, bf2, g2, b2):
    lean = bool(
        not np.any(np.asarray(bo)) and not np.any(np.asarray(bf1))
        and not np.any(np.asarray(bf2)) and not np.any(np.asarray(b1))
        and not np.any(np.asarray(b2))
        and np.all(np.asarray(g1) == 1.0) and np.all(np.asarray(g2) == 1.0)
    )
    in_maps = _in_maps(
        x, Wv, Wo, bo, g1, b1, Wf1, bf1, Wf2, bf2, g2, b2, lean=lean)
    res = run_bass_kernel_spmd(_get_nc(lean), in_maps, list(range(8))).results
    return _assemble(res)
